# revision 1
# baseline (speedup 1.0000x reference)
"""Trainium2 Bass kernel for nn_CircuitModel (soft sequential XOR circuit).

Math: with u = 1 - 2*s (s = register value), soft-XOR becomes a pure product:
    u_new = u_a * u_b,   u_x = -tanh(2*clip(p, -2, 2))
Magnitudes and signs decouple and both evolve *linearly*, so the whole
64-step scan unrolls at build time (wa/wb known when kernel() is called):
    log|u_final[j]| = sum_k C[j,k] * log|u_x[k]|        (k = (input i, time t))
    sign(u_final[j]) = (-1)^( sum_k D[j,k] * [u_x[k] > 0]  mod 2 ),  D = C mod 2
    u_final[j] = 0 exactly if j's dependency tree reaches the t=0 init state
                 (init s=0.5 -> u=0), or underflows (huge leaf count).
Registers whose tree hits init or has leaf-count >= 4096 output exactly 0.5.

For the instance at hand only ~209 of 256 registers are nontrivial, with
~703 used (i,t) leaf columns (t in [44,64)). Sharding: pure batch-parallel,
512 batch per core; while sharding, the host also packs exactly the used
(i,t) cells into a [128, nchunks, 512] bf16 slot layout (the selection
pattern depends only on wa/wb). The device kernel per core:
  1. DMA the compact slots + the C/D weight matrices into SBUF
  2. ACT/DVE: t = tanh(2p); q = t^2; l = max(ln(q), -88)  (the 1/2 of
     ln|tanh| is folded into the C weights); v = [p > 0]
  3. two accumulating matmul chains over slot chunks:
     L = (C/2)^T l (bf16), Y = D^T v (fp8, exact integer counts)
  4. sign = (-1)^(Y mod 2) via the fp32 round-to-nearest magic constant,
     s = 0.5 - 0.5 * sign * exp(L)
"""

import sys
from contextlib import ExitStack

import numpy as np

sys.path.insert(0, "/opt/trn_rl_repo")

import concourse.mybir as mybir  # noqa: E402
import concourse.tile as tile  # noqa: E402
from concourse import bacc, bass_utils  # noqa: E402

N_IN = 256
N_REG = 256
T = 64
B = 4096
NCORES = 8
BL = B // NCORES  # 512 batch per core
W_CAP = 4096  # leaf-count threshold beyond which u underflows to 0 -> s = 0.5

AF = mybir.ActivationFunctionType
MAGIC = float(1 << 23)  # 2^23: fp32 round-to-nearest-integer magic
ALU = mybir.AluOpType


def _unroll(wa, wb):
    """Exact symbolic unroll of the 64-step recurrence.

    Returns (C counts int64 [N_REG, N_IN*T] saturating, D parity uint8,
    Z bool: u == 0 exactly because the tree reaches the init state)."""
    NC = N_IN * T
    C = np.zeros((N_REG, NC), np.int64)
    Dp = np.zeros((N_REG, NC), np.uint8)
    Z = np.ones(N_REG, bool)
    wa = np.asarray(wa).astype(np.int64)
    wb = np.asarray(wb).astype(np.int64)
    for t in range(T):
        nC = np.zeros_like(C)
        nD = np.zeros_like(Dp)
        nZ = np.zeros(N_REG, bool)
        for src in (wa, wb):
            is_x = src < N_IN
            xrows = np.nonzero(is_x)[0]
            nC[xrows, src[xrows] * T + t] += 1
            nD[xrows, src[xrows] * T + t] ^= 1
            rrows = np.nonzero(~is_x)[0]
            ri = src[rrows] - N_IN
            nC[rrows] += C[ri]
            nD[rrows] ^= Dp[ri]
            nZ[rrows] |= Z[ri]
        np.minimum(nC, 1 << 20, out=nC)
        C, Dp, Z = nC, nD, nZ
    return C, Dp, Z


def _build_plan(wa, wb):
    C, Dp, Z = _unroll(wa, wb)
    W = C.sum(1)
    alive = (~Z) & (W < W_CAP)
    aidx = np.nonzero(alive)[0]
    A = int(len(aidx))
    if A == 0:
        return {"A": 0, "aidx": aidx}
    Ca = C[aidx]
    Da = Dp[aidx]
    used = (Ca != 0).any(0)
    cols = np.nonzero(used)[0]  # flattened (i*T + t) indices of used cells
    ncols = int(len(cols))
    nchunks = (ncols + 127) // 128
    nslots = nchunks * 128
    # pad slots duplicate the first used cell; their C/D columns stay zero
    slot_cols = np.concatenate([cols, np.full(nslots - ncols, cols[0], np.int64)])
    slot_i = slot_cols // T
    slot_t = slot_cols % T

    n_mt = (A + 127) // 128
    Apad = n_mt * 128
    # lhsT layout [slot-in-chunk (K), chunk * Apad + alive-row (M)]
    cw = np.zeros((128, nchunks * Apad), np.float32)
    dw = np.zeros((128, nchunks * Apad), np.float32)
    for s in range(ncols):
        ci = slot_cols[s]
        c, k = divmod(s, 128)
        cw[k, c * Apad : c * Apad + A] = Ca[:, ci] * 0.5
        dw[k, c * Apad : c * Apad + A] = Da[:, ci]
    return {
        "A": A,
        "aidx": aidx,
        "slot_i": slot_i,
        "slot_t": slot_t,
        "nchunks": nchunks,
        "n_mt": n_mt,
        "Apad": Apad,
        "cw": cw,
        "dw": dw,
    }


def _build_nc(plan, reps=1):
    """reps > 1 repeats the whole compute in one NEFF (for benchmarking the
    steady-state per-iteration device time via a two-point slope)."""
    f32 = mybir.dt.float32
    bf16 = mybir.dt.bfloat16
    f8 = mybir.dt.float8e4
    nchunks, n_mt, Apad = plan["nchunks"], plan["n_mt"], plan["Apad"]

    nc = bacc.Bacc("TRN2", debug=False)
    # [slot-in-chunk, chunk, batch] bf16, packed on host while sharding
    p_d = nc.dram_tensor("p_used", [128, nchunks, BL], bf16, kind="ExternalInput")
    cw_d = nc.dram_tensor("cw", [128, nchunks * Apad], bf16, kind="ExternalInput")
    dw_d = nc.dram_tensor("dw", [128, nchunks * Apad], f8, kind="ExternalInput")
    out_d = nc.dram_tensor("outs", [Apad, BL], f32, kind="ExternalOutput")

    with tile.TileContext(nc) as tc, ExitStack() as ctx:
        pool = ctx.enter_context(tc.tile_pool(name="pool", bufs=1))
        tmp = ctx.enter_context(tc.tile_pool(name="tmp", bufs=2))
        mps = ctx.enter_context(tc.tile_pool(name="mps", bufs=1, space="PSUM"))

        cw_s = pool.tile([128, nchunks * Apad], bf16)
        nc.sync.dma_start(cw_s[:], cw_d[:])
        dw_s = pool.tile([128, nchunks * Apad], f8)
        nc.sync.dma_start(dw_s[:], dw_d[:])

        SLAB = 3  # chunks per elementwise op (amortizes per-op overheads)
        slabs = [
            (s0, min(s0 + SLAB, nchunks)) for s0 in range(0, nchunks, SLAB)
        ]
        for rep in range(reps):
            stg = pool.tile([128, nchunks * BL], bf16, name="stg", tag="stg")
            nc.sync.dma_start(stg[:], p_d.rearrange("k c b -> k (c b)"))

            l_s = pool.tile([128, nchunks * BL], bf16, name="l_s", tag="l_s")
            v_s = pool.tile([128, nchunks * BL], f8, name="v_s", tag="v_s")
            q_s = pool.tile([128, nchunks * BL], f32, name="q_s", tag="q_s")
            L_ps = mps.tile([128, n_mt * BL], f32, name="Lp", tag="Lp")
            Y_ps = mps.tile([128, n_mt * BL], f32, name="Yp", tag="Yp")

            # phase A (ACT table set exp_and_others: tanh): q = tanh(2p)^2 and
            # the parity operand v = [p > 0], plus the exact fp8 parity matmuls
            for s0, s1 in slabs:
                sl = slice(s0 * BL, s1 * BL)
                t_t = tmp.tile([128, SLAB * BL], f32, tag="t1")
                tw = t_t[:, 0 : (s1 - s0) * BL]
                nc.scalar.activation(tw, stg[:, sl], AF.Tanh, scale=2.0)
                nc.vector.tensor_tensor(q_s[:, sl], tw, tw, ALU.mult)
                nc.vector.tensor_scalar(v_s[:, sl], stg[:, sl], 0.0, None, ALU.is_gt)
                for c in range(s0, s1):
                    for mt in range(n_mt):
                        nc.tensor.matmul(
                            Y_ps[:, mt * BL : (mt + 1) * BL],
                            dw_s[:, c * Apad + mt * 128 : c * Apad + (mt + 1) * 128],
                            v_s[:, c * BL : (c + 1) * BL],
                            start=(c == 0),
                            stop=(c == nchunks - 1),
                        )

            # phase B (ACT table set natural_log_exp_and_others): l =
            # max(ln(q), -88) in bf16 and the magnitude matmul chain
            for s0, s1 in slabs:
                sl = slice(s0 * BL, s1 * BL)
                lr = tmp.tile([128, SLAB * BL], f32, tag="lr")
                lw = lr[:, 0 : (s1 - s0) * BL]
                nc.scalar.activation(lw, q_s[:, sl], AF.Ln)
                nc.vector.tensor_scalar_max(l_s[:, sl], lw, -88.0)
                for c in range(s0, s1):
                    for mt in range(n_mt):
                        nc.tensor.matmul(
                            L_ps[:, mt * BL : (mt + 1) * BL],
                            cw_s[:, c * Apad + mt * 128 : c * Apad + (mt + 1) * 128],
                            l_s[:, c * BL : (c + 1) * BL],
                            start=(c == 0),
                            stop=(c == nchunks - 1),
                        )

            # postamble over the combined [128, n_mt*BL] tiles:
            #   e = exp(L); parity of Y via round-to-nearest magic:
            #   h = RN(Y/2); d = Y/2 - h in {0, +-1/2}; a = |d|
            #   sigma = 1 - 4a;  s = 0.5 - 0.5*sigma*e = 2*(a - 1/4)*e + 0.5
            # (parity arithmetic on DVE to balance the engines; ACT keeps the
            # LUT ops)
            e_t = tmp.tile([128, n_mt * BL], f32, tag="p1")
            nc.scalar.activation(e_t[:], L_ps[:], AF.Exp)
            g_t = tmp.tile([128, n_mt * BL], f32, tag="p2")
            nc.vector.tensor_scalar(g_t[:], Y_ps[:], 0.5, MAGIC, ALU.mult, ALU.add)
            h_t = tmp.tile([128, n_mt * BL], f32, tag="p2b")
            nc.vector.tensor_scalar_sub(h_t[:], g_t[:], MAGIC)
            d_t = tmp.tile([128, n_mt * BL], f32, tag="p3")
            nc.vector.scalar_tensor_tensor(
                d_t[:], Y_ps[:], 0.5, h_t[:], ALU.mult, ALU.subtract
            )
            a_t = tmp.tile([128, n_mt * BL], f32, tag="p4")
            nc.scalar.activation(a_t[:], d_t[:], AF.Abs)
            y_t = tmp.tile([128, n_mt * BL], f32, tag="p5")
            nc.vector.scalar_tensor_tensor(
                y_t[:], a_t[:], -0.25, e_t[:], ALU.add, ALU.mult
            )
            s_t2 = tmp.tile([128, n_mt * BL], f32, tag="p7")
            nc.scalar.activation(s_t2[:], y_t[:], AF.Copy, scale=2.0, bias=0.5)
            nc.sync.dma_start(
                out_d.rearrange("(m k) b -> k m b", m=n_mt),
                s_t2.rearrange("k (m b) -> k m b", m=n_mt),
            )

    nc.compile()
    return nc


_CACHE = {}


def _get_compiled(wa, wb):
    key = (np.asarray(wa).tobytes(), np.asarray(wb).tobytes())
    if key not in _CACHE:
        plan = _build_plan(wa, wb)
        nc = _build_nc(plan) if plan["A"] > 0 else None
        _CACHE[key] = (plan, nc)
    return _CACHE[key]


def _pack_core(P, plan, c):
    """Pack core c's compact slot tensor [128, nchunks, BL] bf16."""
    bf = mybir.dt.np(mybir.dt.bfloat16)
    sel = P[plan["slot_i"], c * BL : (c + 1) * BL, plan["slot_t"]]  # [nslots, BL]
    nchunks = plan["nchunks"]
    return np.ascontiguousarray(
        sel.reshape(nchunks, 128, BL).transpose(1, 0, 2)
    ).astype(bf)


def run(P, wa, wb, trace=False):
    """Returns (out [B, N_REG] float32, BassKernelResults-or-None)."""
    P = np.asarray(P)
    plan, nc = _get_compiled(wa, wb)
    out = np.full((B, N_REG), 0.5, np.float32)
    if plan["A"] == 0:
        return out, None

    bf = mybir.dt.np(mybir.dt.bfloat16)
    f8 = mybir.dt.np(mybir.dt.float8e4)
    cw = plan["cw"].astype(bf)
    dw = plan["dw"].astype(f8)
    in_maps = [
        {"p_used": _pack_core(P, plan, c), "cw": cw, "dw": dw}
        for c in range(NCORES)
    ]

    res = bass_utils.run_bass_kernel_spmd(
        nc, in_maps, list(range(NCORES)), trace=trace
    )
    A = plan["A"]
    aidx = plan["aidx"]
    for c in range(NCORES):
        s_core = np.asarray(res.results[c]["outs"])  # [Apad, BL]
        out[c * BL : (c + 1) * BL, aidx] = s_core[:A].T
    return out, res


def kernel(P, wa, wb):
    out, _ = run(P, wa, wb, trace=False)
    return out

